# revision 9
# baseline (speedup 1.0000x reference)
"""CrossSigmoidFocalLoss Trainium2 kernel.

Computes mean over (N=262144, C=80) of
    focal_w * bce * (weight>0) * cross_mask
where
    focal_w = (0.25*oh + 0.75*(1-oh)) * pt^2,  pt = oh*(1-p) + (1-oh)*p
    bce     = oh*softplus(-x) + (1-oh)*softplus(x)
    oh      = one_hot(targets, 80)  (targets==80 -> all-zero row)
    cross_mask = bit_c(int(weight)) on background rows (targets==80), else 1.
weight < 2**16 so only bits 0..15 can be set.

Strategy (per core, 8-way row sharding, 32768 rows each):
  All-negative base field: pe = p^2 * ln(1-p)  (<=0), p = sigmoid(x).
  Row sums weighted by facA = 0.75*(w>0)*(t!=80) via per-group PE matmuls
  ([128,1] x [128,80]) accumulating into one PSUM [1,80].
  Background rows: peB = pe[:, c<16]*bit_c, weighted by facB via PE into [1,16].
  Positive-column correction per non-bg row r (at c=t_r):
      corr = w * (0.25*(1-p_t)^2*(-ln p_t) - 0.75*p_t^2*(-ln(1-p_t)))
  with p_t gathered by a fused one-hot scalar_tensor_tensor (accum_out).
  loss_total = -(psum_neg + psum_bg) + corr_total ; host divides by N*C.
"""

import numpy as np

import concourse.bass as bass
import concourse.bacc as bacc
import concourse.tile as tile
from concourse import mybir
from concourse.bass_utils import run_bass_kernel_spmd

F32 = mybir.dt.float32
BF16 = mybir.dt.bfloat16
I32 = mybir.dt.int32
ALU = mybir.AluOpType
AFT = mybir.ActivationFunctionType

N_CORES = 8
N = 262144
C = 80
R = N // N_CORES          # 32768 rows per core
P = 128                   # partitions
A = R // P                # 256 row-columns per partition (row = p*A + a)
G = 32                    # row-groups per big tile
T_TILES = A // G          # 8 big tiles
FD = G * C                # 2560 free elems per big tile
NBITS = 16                # weight < 2**16


def build_kernel() -> bass.Bass:
    nc = bacc.Bacc()
    pred = nc.dram_tensor("pred", [R, C], F32, kind="ExternalInput")
    targets = nc.dram_tensor("targets", [R], I32, kind="ExternalInput")
    weight = nc.dram_tensor("weight", [R], F32, kind="ExternalInput")
    out = nc.dram_tensor("out", [1, 1], F32, kind="ExternalOutput")

    # row = ((p*T + t)*G + g) ; per tile t the (g c) block is 2560 contiguous f32
    pred_v = pred[:, :].rearrange("(p t g) c -> t p (g c)", p=P, t=T_TILES, g=G)
    targets_v = targets[:].rearrange("(p a) -> p a", p=P)
    weight_v = weight[:].rearrange("(p a) -> p a", p=P)

    with tile.TileContext(nc) as tc:
        with (
            tc.tile_pool(name="singles", bufs=1) as singles,
            tc.tile_pool(name="xin", bufs=3) as xin,
            tc.tile_pool(name="pbuf", bufs=T_TILES) as pbuf,
            tc.tile_pool(name="scr", bufs=2) as scr,
            tc.tile_pool(name="psum", bufs=1, space="PSUM") as psum,
        ):
            # ---------------- constants / row-level setup ----------------
            iota_cls_g = singles.tile([P, C], BF16)
            nc.gpsimd.iota(iota_cls_g, [[1, C]], base=0, channel_multiplier=0,
                           allow_small_or_imprecise_dtypes=True)
            iota_bits_g = singles.tile([P, NBITS], I32)
            nc.gpsimd.iota(iota_bits_g, [[1, NBITS]], base=0, channel_multiplier=0)
            # stage through the vector engine so consumers don't need
            # cross-engine waits on gpsimd (TSP has few sync-wait slots)
            iota_cls = singles.tile([P, C], BF16)
            nc.vector.tensor_copy(out=iota_cls, in_=iota_cls_g)
            iota_bits = singles.tile([P, NBITS], I32)
            nc.vector.tensor_copy(out=iota_bits, in_=iota_bits_g)

            t_i32 = singles.tile([P, A], I32)
            nc.sync.dma_start(out=t_i32, in_=targets_v)
            w_f32 = singles.tile([P, A], F32)
            nc.sync.dma_start(out=w_f32, in_=weight_v)

            t_bf = singles.tile([P, A], BF16)
            nc.vector.tensor_copy(out=t_bf, in_=t_i32)

            # w01 = (w > 0), notbg = (t != 80), facC = w01*notbg
            w01 = singles.tile([P, A], F32)
            nc.vector.tensor_scalar(out=w01, in0=w_f32, scalar1=0.0, scalar2=None,
                                    op0=ALU.is_gt)
            facC = singles.tile([P, A], F32)
            nc.vector.scalar_tensor_tensor(out=facC, in0=t_i32, scalar=C,
                                           in1=w01, op0=ALU.not_equal,
                                           op1=ALU.mult)
            facA = singles.tile([P, A], BF16)
            nc.vector.tensor_scalar(out=facA, in0=facC, scalar1=0.75, scalar2=None,
                                    op0=ALU.mult)
            # facB = 0.75*w01*(t==80) = 0.75*w01 - facA   (in bf16)
            facB = singles.tile([P, A], BF16)
            nc.vector.scalar_tensor_tensor(out=facB, in0=w01, scalar=0.75,
                                           in1=facA, op0=ALU.mult,
                                           op1=ALU.subtract)

            # bits_bf[p, a, c] = bit c of int(weight[p, a]), c < 16
            wi32 = singles.tile([P, A], I32)
            nc.vector.tensor_copy(out=wi32, in_=w_f32)
            zero_i32 = singles.tile([P, 1], I32)
            nc.vector.memset(zero_i32, 0)
            shr = singles.tile([P, A, NBITS], I32)
            wi_b = wi32[:, :].unsqueeze(2).broadcast_to([P, A, NBITS])
            ct_b = iota_bits[:, :].unsqueeze(1).broadcast_to([P, A, NBITS])
            nc.vector.scalar_tensor_tensor(out=shr, in0=wi_b,
                                           scalar=zero_i32[:, 0:1],
                                           in1=ct_b, op0=ALU.bitwise_or,
                                           op1=ALU.logical_shift_right)
            one_i32 = singles.tile([P, 1], I32)
            nc.vector.memset(one_i32, 1)
            bits_i = singles.tile([P, A, NBITS], I32)
            nc.vector.tensor_scalar(out=bits_i, in0=shr,
                                    scalar1=one_i32[:, 0:1], scalar2=None,
                                    op0=ALU.bitwise_and)
            bits_bf = singles.tile([P, A, NBITS], BF16)
            nc.vector.tensor_copy(out=bits_bf, in_=bits_i)

            ones_f32 = singles.tile([P, 1], F32)
            nc.vector.memset(ones_f32, 1.0)

            ptacc = singles.tile([P, A], F32)     # gathered p_t per row

            psum_neg = psum.tile([1, C], F32)
            psum_bg = psum.tile([1, NBITS], F32)
            psum_corr = psum.tile([1, 1], F32)

            # ---------------- phase A: sigmoid + one-hot gather ----------------
            p_tiles = []
            for t in range(T_TILES):
                x_t = xin.tile([P, FD], F32)
                nc.gpsimd.dma_start(out=x_t, in_=pred_v[t])
                p_t = pbuf.tile([P, FD], BF16, tag="p_persist")
                nc.scalar.activation(out=p_t, in_=x_t, func=AFT.Sigmoid)
                p_tiles.append(p_t)

                ohp = scr.tile([P, FD], BF16, tag="ohp")
                for g in range(G):
                    a = t * G + g
                    nc.vector.scalar_tensor_tensor(
                        out=ohp[:, g * C:(g + 1) * C],
                        in0=iota_cls,
                        scalar=t_bf[:, a:a + 1],
                        in1=p_t[:, g * C:(g + 1) * C],
                        op0=ALU.is_equal,
                        op1=ALU.mult,
                        accum_out=ptacc[:, a:a + 1],
                    )

            # ---------------- phase B: ln, pe field, PE reductions ----------------
            first = True
            for t in range(T_TILES):
                p_t = p_tiles[t]
                l1 = scr.tile([P, FD], BF16, tag="l1")
                # ln(1 - p)
                nc.scalar.activation(out=l1, in_=p_t, func=AFT.Ln,
                                     bias=1.0, scale=-1.0)
                q_t = scr.tile([P, FD], BF16, tag="q")
                nc.vector.scalar_tensor_tensor(out=q_t, in0=p_t, scalar=0.0,
                                               in1=p_t, op0=ALU.add,
                                               op1=ALU.mult)
                pe = scr.tile([P, FD], BF16, tag="pe")
                # pe = max(l1, -30) * p^2   (<= 0)
                nc.vector.scalar_tensor_tensor(out=pe, in0=l1, scalar=-30.0,
                                               in1=q_t, op0=ALU.max,
                                               op1=ALU.mult)
                pe3 = pe[:, :].rearrange("p (g c) -> p g c", g=G)
                peB = scr.tile([P, G, NBITS], BF16, tag="peB")
                nc.vector.scalar_tensor_tensor(
                    out=peB, in0=pe3[:, :, 0:NBITS], scalar=0.0,
                    in1=bits_bf[:, t * G:(t + 1) * G, :],
                    op0=ALU.add, op1=ALU.mult)

                for g in range(G):
                    a = t * G + g
                    last = (t == T_TILES - 1) and (g == G - 1)
                    nc.tensor.matmul(psum_neg[:, :], facA[:, a:a + 1],
                                     pe3[:, g, :], start=first, stop=last)
                    nc.tensor.matmul(psum_bg[:, :], facB[:, a:a + 1],
                                     peB[:, g, :], start=first, stop=last)
                    first = False

            # ---------------- phase C: row-level correction ----------------
            ptc = singles.tile([P, A], F32)
            nc.vector.tensor_scalar(out=ptc, in0=ptacc,
                                    scalar1=1e-6, scalar2=1.0 - 2.0 ** -9,
                                    op0=ALU.max, op1=ALU.min)
            ln_pt = singles.tile([P, A], F32)
            nc.scalar.activation(out=ln_pt, in_=ptc, func=AFT.Ln)
            ln_1m = singles.tile([P, A], F32)
            nc.scalar.activation(out=ln_1m, in_=ptc, func=AFT.Ln,
                                 bias=1.0, scale=-1.0)
            one_m = singles.tile([P, A], F32)
            nc.vector.tensor_scalar(out=one_m, in0=ptc, scalar1=-1.0, scalar2=1.0,
                                    op0=ALU.mult, op1=ALU.add)
            sq1m = singles.tile([P, A], F32)
            nc.vector.scalar_tensor_tensor(out=sq1m, in0=one_m, scalar=0.0,
                                           in1=one_m, op0=ALU.add, op1=ALU.mult)
            sqpt = singles.tile([P, A], F32)
            nc.vector.scalar_tensor_tensor(out=sqpt, in0=ptc, scalar=0.0,
                                           in1=ptc, op0=ALU.add, op1=ALU.mult)
            t1 = singles.tile([P, A], F32)
            nc.vector.scalar_tensor_tensor(out=t1, in0=ln_pt, scalar=-0.25,
                                           in1=sq1m, op0=ALU.mult, op1=ALU.mult)
            t2 = singles.tile([P, A], F32)
            nc.vector.scalar_tensor_tensor(out=t2, in0=ln_1m, scalar=0.75,
                                           in1=sqpt, op0=ALU.mult, op1=ALU.mult)
            corrf = singles.tile([P, A], F32)
            nc.vector.tensor_add(out=corrf, in0=t1, in1=t2)
            corrw = singles.tile([P, A], F32)
            corrcol = singles.tile([P, 1], F32)
            nc.vector.scalar_tensor_tensor(out=corrw, in0=corrf, scalar=0.0,
                                           in1=facC, op0=ALU.add, op1=ALU.mult,
                                           accum_out=corrcol)
            nc.tensor.matmul(psum_corr[:, :], corrcol, ones_f32,
                             start=True, stop=True)

            # ---------------- final combine ----------------
            neg_sb = singles.tile([1, C], F32)
            nc.vector.tensor_copy(out=neg_sb, in_=psum_neg)
            bg_sb = singles.tile([1, NBITS], F32)
            nc.vector.tensor_copy(out=bg_sb, in_=psum_bg)
            corr_sb = singles.tile([1, 1], F32)
            nc.vector.tensor_copy(out=corr_sb, in_=psum_corr)
            negtot = singles.tile([1, 1], F32)
            nc.vector.reduce_sum(out=negtot, in_=neg_sb, axis=mybir.AxisListType.X)
            bgtot = singles.tile([1, 1], F32)
            nc.vector.reduce_sum(out=bgtot, in_=bg_sb, axis=mybir.AxisListType.X)
            tot = singles.tile([1, 1], F32)
            nc.vector.tensor_add(out=tot, in0=negtot, in1=bgtot)
            # total = corr - (neg + bg)
            nc.vector.scalar_tensor_tensor(out=tot, in0=tot, scalar=-1.0,
                                           in1=corr_sb, op0=ALU.mult, op1=ALU.add)
            nc.sync.dma_start(out=out[:, :], in_=tot)

    nc.compile()
    return nc


_NC_CACHE = None


def kernel(pred: np.ndarray, targets: np.ndarray, weight: np.ndarray) -> np.ndarray:
    global _NC_CACHE
    if _NC_CACHE is None:
        _NC_CACHE = build_kernel()
    nc = _NC_CACHE

    pred = np.ascontiguousarray(pred, dtype=np.float32)
    targets = np.ascontiguousarray(targets, dtype=np.int32)
    weight = np.ascontiguousarray(weight, dtype=np.float32)

    in_maps = []
    for k in range(N_CORES):
        sl = slice(k * R, (k + 1) * R)
        in_maps.append({
            "pred": pred[sl],
            "targets": targets[sl],
            "weight": weight[sl],
        })
    res = run_bass_kernel_spmd(nc, in_maps, core_ids=list(range(N_CORES)))
    total = sum(float(r["out"][0, 0]) for r in res.results)
    return np.float32(total / (N * C))
